# revision 21
# baseline (speedup 1.0000x reference)
"""Trainium2 Bass kernel for nn_CausalMessagePassingLayer (GNN message passing).

Sharding (8 NeuronCores, B=4 graphs): 2 cores per graph.
  - GCN layer: core pair splits the M=16384 edge-node rows in half (each core
    aggregates the GCN output V for its 8192 destination rows).
  - Graph cross-attention: edges are assigned to the core that owns the
    *source* edge-node row, so the value/key gathers are purely local.  The
    per-destination softmax numerator/denominator partials are combined with
    one pairwise AllReduce.
  - Final gated linear: each core produces its half of the token rows.

All floating point math runs on device.  The host only reshapes/sorts integer
index inputs into DMA-gather index blobs and one-hot scatter tiles (values
derived from graph degrees only).

Irregular ops are expressed as:
  gather:      SWDGE dma_gather (row gather from HBM tables)
  scatter-add: one-hot matmuls accumulating in PSUM (segment sum by dest tile)
"""
import sys

sys.path.insert(0, "/opt/trn_rl_repo")

import numpy as np
import ml_dtypes

import concourse.bacc as bacc
import concourse.tile as tile
import concourse.mybir as mybir
from concourse.masks import make_identity
from bass_rust import ScopedClock

# --- workaround for this walrus build: it encodes at most ONE sync wait per
# instruction, but TileContext's tail drain carries one wait per live
# semaphore.  Spread the waits across a chain of SP nops. ---


def _patched_drain_and_barrier(self, tick_clock, wait_clock):
    nc = self.nc
    probe = nc.sync.nop()
    wait_clock.add_sem_waits(probe.ins, ScopedClock({None: tick_clock.global_clock}))
    si = probe.ins.sync_info
    waits = list(si.on_wait or [])
    if len(waits) > 1:
        si.on_wait = waits[:1]
        for w in waits[1:]:
            extra = nc.sync.nop()
            if extra.ins.sync_info is None:
                extra.ins.sync_info = mybir.SyncInfo(on_wait=[w], on_update=[])
            else:
                extra.ins.sync_info.on_wait = [w]
    nc.sync.drain()
    nc.all_engine_barrier()
    assert self.sems is not None
    popped = nc._tile_sem_poison_stack.pop()
    assert popped is self._sem_poison
    nc.clear_and_free_semaphores(list(self.sems.allocated().values()))
    nc.all_engine_barrier()


tile.TileContext._drain_and_barrier = _patched_drain_and_barrier

f32 = mybir.dt.float32
bf16 = mybir.dt.bfloat16
i16 = mybir.dt.int16
bfl = ml_dtypes.bfloat16

# Problem constants (hardcoded per the contract).
B, N, M, Eg, Ea, D, DK = 4, 4096, 16384, 65536, 65536, 512, 64
NCORES = 8
GROUPS = [[0, 1], [2, 3], [4, 5], [6, 7]]


def _derive():
    global MH, NH, NT_G, NT_A, ROW
    MH = M // 2  # GCN dst rows per core
    NH = N // 2  # output token rows per core
    NT_G = MH // 128  # GCN dst tiles per core
    NT_A = N // 128  # attention dst tiles per core
    ROW = D + 2 * DK  # 640: [V bf16 512 | k bf16 64 | pad 64]


def _set_sizes(b=4, n=4096, m=16384, eg=65536, ea=65536):
    """Test hook: shrink the problem for fast compiles."""
    global B, N, M, Eg, Ea
    B, N, M, Eg, Ea = b, n, m, eg, ea
    _derive()


_derive()


# ---------------------------------------------------------------------------
# Host-side integer index preprocessing
# ---------------------------------------------------------------------------

def _ceil(a, b):
    return -(-a // b)


def _wrap_idx16(idx):
    """dma_gather index wrapping: linear position e -> [e % 16, e // 16]."""
    return np.ascontiguousarray(idx.reshape(-1, 16).T).astype(np.int16)


class EdgePlan:
    """Per-tile-slot edge layout shared by all cores (slot sizes = max over
    cores, padded to a multiple of 32)."""

    def __init__(self, n_tiles, per_core_tiles, pad=32):
        # per_core_tiles: list over cores of list over tiles of
        # (gidx array, dstl array, val array)
        self.n_tiles = n_tiles
        counts = np.zeros((len(per_core_tiles), n_tiles), np.int64)
        for c, tiles in enumerate(per_core_tiles):
            for t, (gi, _, _) in enumerate(tiles):
                counts[c, t] = len(gi)
        kslot = np.maximum(counts.max(axis=0), pad)
        self.kslot = (_ceil(kslot, pad) * pad).astype(np.int64)
        self.groups = _ceil(self.kslot, 128)
        self.goff = np.concatenate([[0], np.cumsum(self.groups)])  # group offsets
        self.ioff = np.concatenate([[0], np.cumsum(self.kslot // 16)])  # idx16 cols
        self.total_groups = int(self.goff[-1])
        self.total_icols = int(self.ioff[-1])

    def blobs(self, tiles):
        """Build (idx16 [128, total_icols], sel [128, total_groups*128]) for one
        core from its per-tile (gidx, dstl, val)."""
        idx16 = np.zeros((16, self.total_icols), np.int16)
        sel = np.zeros((128, self.total_groups * 128), bfl)
        for t, (gi, dstl, val) in enumerate(tiles):
            k = self.kslot[t]
            gip = np.zeros(k, np.int64)
            gip[: len(gi)] = gi
            idx16[:, self.ioff[t]: self.ioff[t + 1]] = _wrap_idx16(gip)
            if len(gi):
                e = np.arange(len(gi))
                p = e % 128
                col = self.goff[t] * 128 + (e // 128) * 128 + dstl
                sel[p, col] = val.astype(bfl)
        return np.tile(idx16, (8, 1)), sel


def host_prep(inputs):
    """Returns per-core input dicts (minus the device-built shapes) and the two
    EdgePlans (GCN, attention)."""
    t2e = np.asarray(inputs["tokens2edges"]).astype(np.int64)  # [B, M]
    ei = np.asarray(inputs["edge_index"]).astype(np.int64)  # [B, 2, Eg]
    esrc = np.asarray(inputs["e2t_src"]).astype(np.int64)  # [B, Ea]
    edst = np.asarray(inputs["e2t_dst"]).astype(np.int64)  # [B, Ea]

    gcn_tiles = [[] for _ in range(NCORES)]
    att_tiles = [[] for _ in range(NCORES)]
    for b in range(B):
        src, dst = ei[b, 0], ei[b, 1]
        deg = np.bincount(dst, minlength=M).astype(np.float64) + 1.0
        dinv = 1.0 / np.sqrt(deg)
        es = np.concatenate([src, np.arange(M)])
        ed = np.concatenate([dst, np.arange(M)])
        gsrc = t2e[b][es]  # token index per GCN edge
        norm = (dinv[es] * dinv[ed]).astype(np.float32)
        for h in range(2):
            c = 2 * b + h
            m = (ed // MH) == h
            e_ed = ed[m] - h * MH
            e_gs = gsrc[m]
            e_no = norm[m]
            tid = e_ed // 128
            order = np.argsort(tid, kind="stable")
            e_ed, e_gs, e_no, tid = e_ed[order], e_gs[order], e_no[order], tid[order]
            bounds = np.searchsorted(tid, np.arange(NT_G + 1))
            for t in range(NT_G):
                s, e = bounds[t], bounds[t + 1]
                gcn_tiles[c].append((e_gs[s:e], e_ed[s:e] % 128, e_no[s:e]))

        asrc, adst = esrc[b], edst[b]
        for h in range(2):
            c = 2 * b + h
            m = (asrc // MH) == h
            a_s = asrc[m] - h * MH
            a_d = adst[m]
            tid = a_d // 128
            order = np.argsort(tid, kind="stable")
            a_s, a_d, tid = a_s[order], a_d[order], tid[order]
            bounds = np.searchsorted(tid, np.arange(NT_A + 1))
            for t in range(NT_A):
                s, e = bounds[t], bounds[t + 1]
                att_tiles[c].append(
                    (a_s[s:e], a_d[s:e] % 128, np.ones(e - s, np.float32), a_d[s:e])
                )

    gplan = EdgePlan(NT_G, gcn_tiles)
    aplan = EdgePlan(NT_A, [[x[:3] for x in tl] for tl in att_tiles], pad=128)

    te = np.asarray(inputs["token_embeddings"]).astype(np.float32)
    wnames = ["W_gnn", "b_gnn", "Wk", "bk", "Wq", "bq", "Wl", "bl", "gate_a", "gate_b"]
    in_maps = []
    for c in range(NCORES):
        b = c // 2
        gi, gsel = gplan.blobs(gcn_tiles[c])
        ai, apatt = aplan.blobs([x[:3] for x in att_tiles[c]])
        # q-gather indices: same layout as attention VK gather, values = dst.
        qi = np.zeros((16, aplan.total_icols), np.int16)
        for t in range(NT_A):
            k = aplan.kslot[t]
            qq = np.zeros(k, np.int64)
            qq[: len(att_tiles[c][t][3])] = att_tiles[c][t][3]
            qi[:, aplan.ioff[t]: aplan.ioff[t + 1]] = _wrap_idx16(qq)
        h = c % 2
        m = {
            "t_in": te[b],
            "t_half": np.ascontiguousarray(te[b][h * NH:(h + 1) * NH]),
            "gcn_idx": gi,
            "gcn_sel": gsel,
            "att_idx": ai,
            "att_patt": apatt,
            "att_qidx": np.tile(qi, (8, 1)),
        }
        for w in wnames:
            m[w] = np.asarray(inputs[w]).astype(np.float32)
        m["gate_a"] = m["gate_a"].reshape(1, 1)
        m["gate_b"] = m["gate_b"].reshape(1, 1)
        m["b_gnn"] = m["b_gnn"].reshape(1, D)
        m["bk"] = m["bk"].reshape(1, DK)
        m["bq"] = m["bq"].reshape(1, DK)
        m["bl"] = m["bl"].reshape(1, D)
        in_maps.append(m)
    return in_maps, gplan, aplan


# ---------------------------------------------------------------------------
# Device kernel builder
# ---------------------------------------------------------------------------

def build_kernel(gplan, aplan):
    nc = bacc.Bacc(None, target_bir_lowering=False, num_devices=NCORES)

    t_in = nc.dram_tensor("t_in", [N, D], f32, kind="ExternalInput")
    t_half = nc.dram_tensor("t_half", [NH, D], f32, kind="ExternalInput")
    W_gnn = nc.dram_tensor("W_gnn", [D, D], f32, kind="ExternalInput")
    b_gnn = nc.dram_tensor("b_gnn", [1, D], f32, kind="ExternalInput")
    Wk = nc.dram_tensor("Wk", [D, DK], f32, kind="ExternalInput")
    bk = nc.dram_tensor("bk", [1, DK], f32, kind="ExternalInput")
    Wq = nc.dram_tensor("Wq", [D, DK], f32, kind="ExternalInput")
    bq = nc.dram_tensor("bq", [1, DK], f32, kind="ExternalInput")
    Wl = nc.dram_tensor("Wl", [D, D], f32, kind="ExternalInput")
    bl = nc.dram_tensor("bl", [1, D], f32, kind="ExternalInput")
    gate_a = nc.dram_tensor("gate_a", [1, 1], f32, kind="ExternalInput")
    gate_b = nc.dram_tensor("gate_b", [1, 1], f32, kind="ExternalInput")
    gcn_idx = nc.dram_tensor("gcn_idx", [128, gplan.total_icols], i16, kind="ExternalInput")
    gcn_sel = nc.dram_tensor("gcn_sel", [128, gplan.total_groups * 128], bf16, kind="ExternalInput")
    att_idx = nc.dram_tensor("att_idx", [128, aplan.total_icols], i16, kind="ExternalInput")
    att_qidx = nc.dram_tensor("att_qidx", [128, aplan.total_icols], i16, kind="ExternalInput")
    att_patt = nc.dram_tensor("att_patt", [128, aplan.total_groups * 128], bf16, kind="ExternalInput")
    y = nc.dram_tensor("y", [NH, D], f32, kind="ExternalOutput")

    TB = nc.dram_tensor("TB", [N, D], bf16, kind="Internal")
    TWX = nc.dram_tensor("TWX", [N, ROW], bf16, kind="Internal")
    QT = nc.dram_tensor("QT", [N, 128], bf16, kind="Internal")
    VK = nc.dram_tensor("VK", [MH, ROW], bf16, kind="Internal")
    CC_IN = nc.dram_tensor("CC_IN", [N, D + 1], f32, kind="Internal")
    CC_OUT = nc.dram_tensor("CC_OUT", [NH, D + 1], f32, kind="Internal")
    NEWB = nc.dram_tensor("NEWB", [NH, D], bf16, kind="Internal")

    with tile.TileContext(nc) as tc:
        _build_body(nc, tc, locals(), gplan, aplan)
    nc.compile()
    return nc


GATHER_CHUNK = 512


def _chunked_gather(nc, out3, table, idxtile, i0, i1, kslot, elem):
    """dma_gather split into <=GATHER_CHUNK-index calls (descriptor-ring cap).
    out3: [128, G, elem] SBUF AP; idx cols [i0, i1) hold kslot/16 columns."""
    done = 0
    while done < kslot:
        n = min(GATHER_CHUNK, kslot - done)
        assert done % 128 == 0 or n == kslot - done
        g0 = done // 128
        g1 = (done + n + 127) // 128
        nc.gpsimd.dma_gather(
            out3[:, g0:g1, :], table, idxtile[:, i0 + done // 16: i0 + (done + n) // 16],
            n, n, elem,
        )
        done += n


def _build_body(nc, tc, T, gplan, aplan):
    import os
    PHASES = int(os.environ.get("KPHASES", "9"))

    def hbm(name):
        return T[name]

    KC = D // 128  # 4 contraction chunks

    with tc.tile_pool(name="res", bufs=1) as res:
        # ---- constants / small residents ----
        ident = res.tile([128, 128], bf16)
        make_identity(nc, ident[:])
        ones_col = res.tile([128, 1], bf16)
        nc.vector.memset(ones_col[:], 1.0)
        ones_row = res.tile([1, 128], bf16)
        nc.vector.memset(ones_row[:], 1.0)

        # gates -> tanh, broadcast to [128,1]
        ga = res.tile([128, 1], f32)
        gb = res.tile([128, 1], f32)
        ga_in = res.tile([1, 1], f32)
        gb_in = res.tile([1, 1], f32)
        nc.sync.dma_start(ga_in[:], hbm("gate_a")[:])
        nc.sync.dma_start(gb_in[:], hbm("gate_b")[:])
        nc.gpsimd.partition_broadcast(ga[:], ga_in[:])
        nc.gpsimd.partition_broadcast(gb[:], gb_in[:])
        nc.scalar.activation(ga[:], ga[:], mybir.ActivationFunctionType.Tanh)
        nc.scalar.activation(gb[:], gb[:], mybir.ActivationFunctionType.Tanh)

        # biases
        bgnn_row = res.tile([1, D], bf16)
        bk_row = res.tile([1, DK], bf16)
        bq_row = res.tile([1, DK], bf16)
        bl_row = res.tile([1, D], bf16)
        tmp_row = res.tile([1, D], f32)
        for name, dstt, w in (("b_gnn", bgnn_row, D), ("bk", bk_row, DK),
                              ("bq", bq_row, DK), ("bl", bl_row, D)):
            nc.sync.dma_start(tmp_row[:1, :w], hbm(name)[:])
            nc.vector.tensor_copy(dstt[:], tmp_row[:1, :w])

        # partition-broadcast bias tiles
        bgnn_bc = res.tile([128, D], bf16, name="bgnn_bc")
        nc.gpsimd.partition_broadcast(bgnn_bc[:], bgnn_row[:])
        bq_bc = res.tile([128, DK], bf16, name="bq_bc")
        nc.gpsimd.partition_broadcast(bq_bc[:], bq_row[:])

        # weights (bf16 chunks, D on partitions)
        wg = [res.tile([128, D], bf16, tag=f"wg{i}", name=f"wg{i}") for i in range(KC)]
        wl = [res.tile([128, D], bf16, tag=f"wl{i}", name=f"wl{i}") for i in range(KC)]
        wkb = [res.tile([128, DK], bf16, tag=f"wkb{i}", name=f"wkb{i}") for i in range(KC)]
        wqb = [res.tile([128, DK], bf16, tag=f"wqb{i}", name=f"wqb{i}") for i in range(KC)]
        with tc.tile_pool(name="wload", bufs=2) as wload:
            for kc in range(KC):
                tmp = wload.tile([128, D], f32)
                nc.sync.dma_start(tmp[:], hbm("W_gnn")[kc * 128:(kc + 1) * 128, :])
                nc.vector.tensor_copy(wg[kc][:], tmp[:])
                tmp2 = wload.tile([128, D], f32)
                nc.sync.dma_start(tmp2[:], hbm("Wl")[kc * 128:(kc + 1) * 128, :])
                nc.vector.tensor_copy(wl[kc][:], tmp2[:])
                tmp3 = wload.tile([128, DK], f32)
                nc.sync.dma_start(tmp3[:], hbm("Wk")[kc * 128:(kc + 1) * 128, :])
                nc.vector.tensor_copy(wkb[kc][:], tmp3[:])
                tmp4 = wload.tile([128, DK], f32)
                nc.sync.dma_start(tmp4[:], hbm("Wq")[kc * 128:(kc + 1) * 128, :])
                nc.vector.tensor_copy(wqb[kc][:], tmp4[:])

        # ---- W_gnn transpose (PE) then W_gnn @ Wk ----
        wgT = [res.tile([128, D], bf16, tag=f"wgT{i}", name=f"wgT{i}") for i in range(KC)]
        with tc.tile_pool(name="tp", bufs=2, space="PSUM") as tpp:
            for i in range(KC):
                for j in range(KC):
                    ps = tpp.tile([128, 128], bf16, space="PSUM")
                    nc.tensor.transpose(ps[:], wg[i][:, j * 128:(j + 1) * 128], ident[:])
                    nc.vector.tensor_copy(wgT[j][:, i * 128:(i + 1) * 128], ps[:])
        wgwk = [res.tile([128, DK], bf16, tag=f"wgwk{i}", name=f"wgwk{i}") for i in range(KC)]
        with tc.tile_pool(name="wk2", bufs=2, space="PSUM") as wk2:
            for j in range(KC):
                ps = wk2.tile([128, DK], f32, space="PSUM")
                for kc in range(KC):
                    nc.tensor.matmul(ps[:], wgT[kc][:, j * 128:(j + 1) * 128], wkb[kc][:],
                                     start=(kc == 0), stop=(kc == KC - 1))
                nc.vector.tensor_copy(wgwk[j][:], ps[:])

        # bk' = b_gnn @ Wk + bk  (as [1, DK])
        bkp = res.tile([1, DK], bf16)
        bcol = res.tile([128, KC], f32)
        nc.sync.dma_start(bcol[:], hbm("b_gnn")[:].rearrange("o (c p) -> p (o c)", p=128))
        bcolb = res.tile([128, KC], bf16)
        nc.vector.tensor_copy(bcolb[:], bcol[:])
        with tc.tile_pool(name="bk2", bufs=1, space="PSUM") as bk2:
            ps = bk2.tile([1, DK], f32, space="PSUM")
            for kc in range(KC):
                nc.tensor.matmul(ps[:], bcolb[:, kc:kc + 1], wkb[kc][:],
                                 start=(kc == 0), stop=(kc == KC - 1))
            nc.vector.tensor_tensor(out=bkp[:], in0=ps[:], in1=bk_row[:],
                                    op=mybir.AluOpType.add)
        bkp_bc = res.tile([128, DK], bf16, name="bkp_bc")
        nc.gpsimd.partition_broadcast(bkp_bc[:], bkp[:])

        # ---- cast t to bf16 (TB) ----
        with tc.tile_pool(name="tcast", bufs=3) as tcast:
            for i in range(N // 128):
                tt = tcast.tile([128, D], f32)
                nc.sync.dma_start(tt[:], hbm("t_in")[i * 128:(i + 1) * 128, :])
                tb = tcast.tile([128, D], bf16)
                nc.vector.tensor_copy(tb[:], tt[:])
                nc.sync.dma_start(hbm("TB")[i * 128:(i + 1) * 128, :], tb[:])

        # ---- tT via DMA transpose ----
        tT = [res.tile([128, N], bf16, tag=f"tT{i}", name=f"tT{i}") for i in range(KC)]
        for c in range(KC):
            nc.sync.dma_start_transpose(tT[c][:], hbm("TB")[:, c * 128:(c + 1) * 128])

        # ---- tw / twk / q ----
        with (
            tc.tile_pool(name="twp", bufs=2, space="PSUM") as twp,
            tc.tile_pool(name="qp", bufs=2, space="PSUM") as qp,
            tc.tile_pool(name="tws", bufs=3) as tws,
        ):
            for i in range(N // 128):
                tok = slice(i * 128, (i + 1) * 128)
                ps = twp.tile([128, D], f32, space="PSUM")
                for kc in range(KC):
                    nc.tensor.matmul(ps[:], tT[kc][:, tok], wg[kc][:],
                                     start=(kc == 0), stop=(kc == KC - 1))
                ps2 = qp.tile([128, DK], f32, space="PSUM")
                for kc in range(KC):
                    nc.tensor.matmul(ps2[:], tT[kc][:, tok], wgwk[kc][:],
                                     start=(kc == 0), stop=(kc == KC - 1))
                stage = tws.tile([128, ROW], bf16)
                nc.vector.tensor_copy(stage[:, :D], ps[:])
                nc.vector.tensor_copy(stage[:, D:D + DK], ps2[:])
                nc.gpsimd.memset(stage[:, D + DK:], 0.0)
                nc.sync.dma_start(hbm("TWX")[tok, :], stage[:])

                ps3 = qp.tile([128, DK], f32, space="PSUM")
                for kc in range(KC):
                    nc.tensor.matmul(ps3[:], tT[kc][:, tok], wqb[kc][:],
                                     start=(kc == 0), stop=(kc == KC - 1))
                qstage = tws.tile([128, 128], bf16)
                nc.vector.tensor_tensor(out=qstage[:, :DK], in0=ps3[:],
                                        in1=bq_bc[:],
                                        op=mybir.AluOpType.add)
                nc.gpsimd.memset(qstage[:, DK:], 0.0)
                nc.sync.dma_start(hbm("QT")[tok, :], qstage[:])

        if PHASES < 1:
            return
        # ---- GCN: per dst tile, gather + one-hot matmuls ----
        gidx_t = res.tile([128, gplan.total_icols], i16)
        nc.sync.dma_start(gidx_t[:], hbm("gcn_idx")[:])
        gmax = int(gplan.groups.max())
        with (
            tc.tile_pool(name="ggather", bufs=3) as ggather,
            tc.tile_pool(name="gsel", bufs=3) as gselp,
            tc.tile_pool(name="gv", bufs=2, space="PSUM") as gvp,
            tc.tile_pool(name="gk", bufs=2, space="PSUM") as gkp,
            tc.tile_pool(name="gst", bufs=3) as gst,
        ):
            for t in range(NT_G):
                kslot = int(gplan.kslot[t])
                G = int(gplan.groups[t])
                gt = ggather.tile([128, gmax, ROW], bf16, tag="ggather")
                _chunked_gather(nc, gt[:, :G, :], hbm("TWX")[:], gidx_t,
                                int(gplan.ioff[t]), int(gplan.ioff[t + 1]), kslot, ROW)
                sel = gselp.tile([128, gmax * 128], bf16, tag="gsel")
                nc.sync.dma_start(
                    sel[:, :G * 128],
                    hbm("gcn_sel")[:, int(gplan.goff[t]) * 128:int(gplan.goff[t + 1]) * 128],
                )
                vps = gvp.tile([128, D], f32, space="PSUM")
                kps = gkp.tile([128, DK], f32, space="PSUM")
                for g in range(G):
                    kg = min(128, kslot - g * 128)
                    lhs = sel[:kg, g * 128:(g + 1) * 128]
                    nc.tensor.matmul(vps[:], lhs, gt[:kg, g, :D],
                                     start=(g == 0), stop=(g == G - 1))
                    nc.tensor.matmul(kps[:], lhs, gt[:kg, g, D:D + DK],
                                     start=(g == 0), stop=(g == G - 1))
                stage = gst.tile([128, ROW], bf16, tag="gstage")
                nc.vector.tensor_tensor(out=stage[:, :D], in0=vps[:],
                                        in1=bgnn_bc[:],
                                        op=mybir.AluOpType.add)
                nc.vector.tensor_tensor(out=stage[:, D:D + DK], in0=kps[:],
                                        in1=bkp_bc[:],
                                        op=mybir.AluOpType.add)
                nc.gpsimd.memset(stage[:, D + DK:], 0.0)
                nc.sync.dma_start(hbm("VK")[t * 128:(t + 1) * 128, :], stage[:])

        if PHASES < 2:
            return
        # ---- Attention: per dst tile ----
        aidx_t = res.tile([128, aplan.total_icols], i16)
        nc.sync.dma_start(aidx_t[:], hbm("att_idx")[:])
        qidx_t = res.tile([128, aplan.total_icols], i16)
        nc.sync.dma_start(qidx_t[:], hbm("att_qidx")[:])
        amax = int(aplan.groups.max())
        with (
            tc.tile_pool(name="agather", bufs=3) as agather,
            tc.tile_pool(name="aq", bufs=3) as aqp,
            tc.tile_pool(name="apatt", bufs=3) as apattp,
            tc.tile_pool(name="aprod", bufs=3) as aprodp,
            tc.tile_pool(name="anum", bufs=2, space="PSUM") as anump,
            tc.tile_pool(name="aden", bufs=2, space="PSUM") as adenp,
            tc.tile_pool(name="ast", bufs=3) as astp,
        ):
            for t in range(NT_A):
                kslot = int(aplan.kslot[t])
                G = int(aplan.groups[t])
                ioff = int(aplan.ioff[t])
                ioff1 = int(aplan.ioff[t + 1])
                import os as _os
                _g = _os.environ.get("KGATE", "vqp")
                gt = agather.tile([128, amax, ROW], bf16, tag="agather")
                if "v" in _g:
                    _chunked_gather(nc, gt[:, :G, :], hbm("VK")[:], aidx_t,
                                    ioff, ioff1, kslot, ROW)
                qg = aqp.tile([128, amax, 128], bf16, tag="aq")
                if "q" in _g:
                    _chunked_gather(nc, qg[:, :G, :], hbm("QT")[:], qidx_t,
                                    ioff, ioff1, kslot, 128)
                patt = apattp.tile([128, amax * 128], bf16, tag="apatt")
                if "p" in _g:
                    nc.sync.dma_start(
                        patt[:, :G * 128],
                        hbm("att_patt")[:, int(aplan.goff[t]) * 128:int(aplan.goff[t + 1]) * 128],
                    )
                SUB = int(os.environ.get("KPH2SUB", "9"))
                if SUB < 1:
                    pack = astp.tile([128, D + 1], f32, tag="apack")
                    nc.gpsimd.memset(pack[:], 0.0)
                    nc.sync.dma_start(hbm("CC_IN")[t * 128:(t + 1) * 128, :], pack[:])
                    continue
                # dots = sum_k qg * kg ; w = exp(dots / 8)
                prod = aprodp.tile([128, amax, DK], f32, tag="aprod")
                nc.vector.tensor_tensor(out=prod[:, :G, :], in0=qg[:, :G, :DK],
                                        in1=gt[:, :G, D:D + DK],
                                        op=mybir.AluOpType.mult)
                dots = aprodp.tile([128, amax], f32, tag="adots")
                nc.vector.tensor_reduce(out=dots[:, :G].unsqueeze(-1),
                                        in_=prod[:, :G, :],
                                        axis=mybir.AxisListType.X,
                                        op=mybir.AluOpType.add)
                wt = aprodp.tile([128, amax], bf16, tag="aw")
                nc.scalar.activation(wt[:, :G], dots[:, :G],
                                     mybir.ActivationFunctionType.Exp, scale=0.125)
                if SUB < 2:
                    pack = astp.tile([128, D + 1], f32, tag="apack")
                    nc.gpsimd.memset(pack[:], 0.0)
                    nc.vector.tensor_copy(pack[:, :1], wt[:, :1])
                    nc.sync.dma_start(hbm("CC_IN")[t * 128:(t + 1) * 128, :], pack[:])
                    continue
                sel = apattp.tile([128, amax * 128], bf16, tag="asel")
                nc.vector.tensor_tensor(
                    out=sel[:, :G * 128].rearrange("p (g j) -> p g j", j=128),
                    in0=patt[:, :G * 128].rearrange("p (g j) -> p g j", j=128),
                    in1=wt[:, :G].unsqueeze(-1).to_broadcast([128, G, 128]),
                    op=mybir.AluOpType.mult,
                )
                if SUB < 3:
                    pack = astp.tile([128, D + 1], f32, tag="apack")
                    nc.gpsimd.memset(pack[:], 0.0)
                    nc.vector.tensor_copy(pack[:, :1], sel[:, :1])
                    nc.sync.dma_start(hbm("CC_IN")[t * 128:(t + 1) * 128, :], pack[:])
                    continue
                nps = anump.tile([128, D], f32, space="PSUM")
                dps = adenp.tile([128, 1], f32, space="PSUM")
                for g in range(G):
                    kg = min(128, kslot - g * 128)
                    lhs = sel[:kg, g * 128:(g + 1) * 128]
                    nc.tensor.matmul(nps[:], lhs, gt[:kg, g, :D],
                                     start=(g == 0), stop=(g == G - 1))
                    nc.tensor.matmul(dps[:], lhs, ones_col[:kg, :],
                                     start=(g == 0), stop=(g == G - 1))
                pack = astp.tile([128, D + 1], f32, tag="apack")
                nc.vector.tensor_copy(pack[:, :D], nps[:])
                nc.vector.tensor_copy(pack[:, D:], dps[:])
                nc.sync.dma_start(hbm("CC_IN")[t * 128:(t + 1) * 128, :], pack[:])

        if PHASES < 3:
            return
        # ---- ReduceScatter num/den partials within the pair: rank h of the
        # pair receives the reduced rows [h*NH, (h+1)*NH) ----
        nc.gpsimd.collective_compute(
            "ReduceScatter", mybir.AluOpType.add, replica_groups=GROUPS,
            ins=[hbm("CC_IN")[:]], outs=[hbm("CC_OUT")[:]],
        )

        if PHASES < 4:
            return
        # ---- final: new = t + tanh(ga)*gatt ; y = new + tanh(gb)*(new@Wl+bl)
        with (
            tc.tile_pool(name="newp", bufs=NH // 128 + 1) as newp,
            tc.tile_pool(name="fin", bufs=3) as fin,
        ):
            new_tiles = []
            for i in range(NH // 128):
                cc = fin.tile([128, D + 1], f32, tag="cc")
                nc.sync.dma_start(cc[:], hbm("CC_OUT")[i * 128:(i + 1) * 128, :])
                tt = fin.tile([128, D], f32, tag="trow")
                nc.sync.dma_start(tt[:], hbm("t_half")[i * 128:(i + 1) * 128, :])
                rc = fin.tile([128, 1], f32, tag="recip")
                nc.vector.tensor_scalar_add(rc[:], cc[:, D:], 1e-30)
                nc.vector.reciprocal(rc[:], rc[:])
                nc.vector.tensor_tensor(out=rc[:], in0=rc[:], in1=ga[:],
                                        op=mybir.AluOpType.mult)
                nt = newp.tile([128, D], f32)
                nc.vector.scalar_tensor_tensor(
                    out=nt[:], in0=cc[:, :D], scalar=rc[:], in1=tt[:],
                    op0=mybir.AluOpType.mult, op1=mybir.AluOpType.add,
                )
                new_tiles.append(nt)
                nb = fin.tile([128, D], bf16, tag="newb")
                nc.scalar.copy(nb[:], nt[:])
                nc.sync.dma_start(hbm("NEWB")[i * 128:(i + 1) * 128, :], nb[:])

            nT = [res.tile([128, NH], bf16, tag=f"nT{i}", name=f"nT{i}") for i in range(KC)]
            for c in range(KC):
                nc.sync.dma_start_transpose(nT[c][:], hbm("NEWB")[:, c * 128:(c + 1) * 128])

            with (
                tc.tile_pool(name="lps", bufs=2, space="PSUM") as lps,
                tc.tile_pool(name="lst", bufs=3) as lst,
            ):
                for i in range(NH // 128):
                    ps = lps.tile([128, D], f32, space="PSUM")
                    for kc in range(KC):
                        nc.tensor.matmul(ps[:], nT[kc][:, i * 128:(i + 1) * 128], wl[kc][:],
                                         start=(kc == 0), stop=False)
                    nc.tensor.matmul(ps[:], ones_row[:], bl_row[:],
                                     start=False, stop=True)
                    ot = lst.tile([128, D], f32, tag="out")
                    nc.vector.scalar_tensor_tensor(
                        out=ot[:], in0=ps[:], scalar=gb[:], in1=new_tiles[i][:],
                        op0=mybir.AluOpType.mult, op1=mybir.AluOpType.add,
                    )
                    nc.sync.dma_start(hbm("y")[i * 128:(i + 1) * 128, :], ot[:])


# ---------------------------------------------------------------------------
# Entry point
# ---------------------------------------------------------------------------

def prepare(inputs):
    in_maps, gplan, aplan = host_prep(inputs)
    nc = build_kernel(gplan, aplan)

    def assemble(results):
        out = np.zeros((B, N, D), np.float32)
        for c in range(NCORES):
            b, h = c // 2, c % 2
            out[b, h * NH:(h + 1) * NH, :] = results[c]["y"]
        return out

    return nc, in_maps, assemble


def kernel(**inputs):
    from concourse.bass_utils import run_bass_kernel_spmd

    nc, in_maps, assemble = prepare(inputs)
    res = run_bass_kernel_spmd(nc, in_maps, core_ids=list(range(NCORES)))
    return assemble(res.results)
